# revision 22
# baseline (speedup 1.0000x reference)
"""Trainium2 Bass kernel for CompressedSparseAttention.

Sharding: 8 cores = 2 batches x 4 interleaved query-chunks. Core (b, j)
handles global query tiles g = 4i + j (i = 0..3), i.e. 512 queries. The
interleave makes causal bounds core-invariant: local tile i only needs the
first 128*(i+1) compressed blocks, so every core skips the same invalid
region. Each core recomputes the (cheap) compressed KV / indexer-K for its
batch from the full H. No collectives.

Layouts (to avoid on-chip transposes of the big attention matrices):
  - S^T [s, q] = (K^T-slice as stationary) @ Q^T   -> softmax masks applied
    elementwise in [s, q]; no P~ transpose needed for PV.
  - PV: out O^T [c, q] accumulates V-stationary matmuls. V is augmented with
    a ones-column, so O^T row 64 is the softmax denominator (free rowsum).
  - O^T is transposed back (16 small PE transposes) for inverse rope and the
    folded output projection sum_h O_h @ A_h + bias.

Perf notes (v2):
  - compressed-KV compressor computed transposed ([C, Tc]) with wide-N
    (512-col) streaming matmuls: 16 matmuls instead of 64.
  - indexer path in f32r (1 cyc/col when N>=256 vs 4 for fp32); indexer
    dots always full-width so f32r streams at full rate. Tie/causal ramp
    comes from a host-built additive table (iacc_tab) instead of on-chip.
  - sliding-band S computed for all 4 heads in one matmul per (i, half):
    batched exp + mask too.
  - comp-attention P~ and V in bf16 (selection already done; only softmax
    weights ride through), band already bf16.
  - output projection packs head pairs onto 128 partitions (2 matmuls).

Top-8 selection uses vector.max + is_ge threshold with an additive ramp
table reproducing jax top_k tie semantics exactly (incl. the -inf leakage
for rows with <8 causal blocks). Softmax needs no row-max: |S|/sqrt(C) <= 8.
"""
import sys

if '/opt/trn_rl_repo' not in sys.path:
    sys.path.insert(0, '/opt/trn_rl_repo')

import numpy as np
import ml_dtypes
import concourse.bass as bass
import concourse.bacc as bacc
import concourse.tile as tile
from concourse import mybir
from concourse.bass_utils import run_bass_kernel_spmd

F32 = mybir.dt.float32
F32R = mybir.dt.float32r
BF16 = mybir.dt.bfloat16
AF = mybir.ActivationFunctionType
ALU = mybir.AluOpType

B, T, D = 2, 2048, 256
C, NH, NWIN = 64, 4, 16
TC = T // 4            # 512 compressed blocks
TQ = 512               # queries per core
NQT = TQ // 128        # 4 query tiles per core
TPAD = T + 4           # H padded for the overlapped compressor windows
NBAND = 1024           # band KV rows per core: 4 disjoint 256-row bands
EPS = 1e-6
DEBUG_SEL = False


def _swap64(ap_slice):
    """AP reading cols [32:64] then [0:32] of a contiguous [P, 64] slice."""
    return bass.AP(tensor=ap_slice.tensor, offset=ap_slice.offset + 32,
                   ap=[ap_slice.ap[0], [-32, 2], [1, 32]])


def _rep(ap_slice, n):
    """AP repeating a [128, W] slice n times along a new leading free dim."""
    return bass.AP(tensor=ap_slice.tensor, offset=ap_slice.offset,
                   ap=[ap_slice.ap[0], [0, n]] + list(ap_slice.ap[1:]))


def build_program():
    nc = bacc.Bacc("TRN2", target_bir_lowering=False, debug=False)

    def din(name, shape, dt=F32):
        return nc.dram_tensor(name, shape, dt, kind="ExternalInput").ap()

    d = {}
    # ordered so early-stage tensors DMA first (round-robin 3 queues)
    d['ht_bf'] = din("ht_bf", [128, 2, TPAD], BF16)  # H[b].T (bf16, KV comp)
    d['wccomp'] = din("wccomp", [128, 16, C], BF16)
    d['ident'] = din("ident", [128, 128])
    d['ht'] = din("ht", [128, 2, TPAD], F32R)        # H[b].T (indexer K)
    d['wcidx'] = din("wcidx", [128, 16, 32], F32R)
    d['cosk'] = din("cosk", [128, 4, 32])            # compressed positions
    d['sinsk'] = din("sinsk", [128, 4, C])
    d['gk_rep'] = din("gk_rep", [128, C])
    d['gv_rep'] = din("gv_rep", [128, C])
    d['htb_bf'] = din("htb_bf", [128, 2, NBAND], BF16)  # band col-blocks
    d['wkv'] = din("wkv", [128, 2, C], BF16)
    d['coskb'] = din("coskb", [128, 8, 32])          # band positions
    d['sinskb'] = din("sinskb", [128, 8, C])
    d['htq'] = din("htq", [128, 2, TQ], F32R)        # own query cols
    d['wdq'] = din("wdq", [128, 2, 64], F32R)
    d['wiuq'] = din("wiuq", [64, 128], F32R)
    d['ww'] = din("ww", [64, 4], F32R)
    d['iacc_tab'] = din("iacc_tab", [128, NQT, TC])  # causal/tie ramp table
    d['wq'] = din("wq", [128, 2, 256], F32R)
    d['cosq'] = din("cosq", [128, NQT, 32])          # cos at query positions
    d['sinsq'] = din("sinsq", [128, NQT, C])         # signed sin [-sin | +sin]
    d['gq_rep'] = din("gq_rep", [128, 256])
    d['smaskT'] = din("smaskT", [128, 2, 2, 128])    # slide masks [s-half, which, q]
    d['a2'] = din("a2", [128, 2, 256], F32R)         # head-pair-packed out proj
    d['bias'] = din("bias", [128, 256])              # folded output bias
    d['ones_vc'] = din("ones_vc", [128, 4, 1], BF16)
    d['ones_vb'] = din("ones_vb", [128, 8, 1], BF16)

    out_d = nc.dram_tensor("out", [128, NQT, 256], F32, kind="ExternalOutput").ap()
    dbg = None
    if DEBUG_SEL:
        dbg = {
            'selmask': nc.dram_tensor("dbg_selmask", [128, NQT, TC], F32,
                                      kind="ExternalOutput").ap(),
            'kit': nc.dram_tensor("dbg_kit", [32, TC], F32,
                                  kind="ExternalOutput").ap(),
            'hdct': nc.dram_tensor("dbg_hdct", [64, TQ], F32,
                                   kind="ExternalOutput").ap(),
            'qit': nc.dram_tensor("dbg_qit", [32, NH, TQ], F32,
                                  kind="ExternalOutput").ap(),
            'wiw': nc.dram_tensor("dbg_wiw", [128, NQT, 4], F32,
                                  kind="ExternalOutput").ap(),
            'iacc': nc.dram_tensor("dbg_iacc", [128, NQT, TC], F32,
                                   kind="ExternalOutput").ap(),
        }

    with tile.TileContext(nc) as tc:
        _build_body(nc, tc, d, out_d, dbg)
    nc.compile()
    return nc


def _build_body(nc, tc, dins, out_d, dbg=None):
    from contextlib import ExitStack
    ctx = ExitStack()
    consts = ctx.enter_context(tc.tile_pool(name="consts", bufs=1))
    persist = ctx.enter_context(tc.tile_pool(name="persist", bufs=1))
    scr = ctx.enter_context(tc.tile_pool(name="scr", bufs=2))
    scr_big = ctx.enter_context(tc.tile_pool(name="scr_big", bufs=2))
    ps_mid = ctx.enter_context(tc.tile_pool(name="ps_mid", bufs=2, space="PSUM"))
    ps_big = ctx.enter_context(tc.tile_pool(name="ps_big", bufs=2, space="PSUM"))
    ps_tp = ctx.enter_context(tc.tile_pool(name="ps_tp", bufs=2, space="PSUM"))
    ps_ot = ctx.enter_context(tc.tile_pool(name="ps_ot", bufs=2, space="PSUM"))

    cdt = {'ht': F32R, 'ht_bf': BF16, 'htq': F32R, 'htb_bf': BF16,
           'wdq': F32R, 'wiuq': F32R, 'ww': F32R, 'wcidx': F32R,
           'wccomp': BF16, 'wkv': BF16, 'wq': F32R, 'a2': F32R}
    # explicit DMA rings ordered by need time; small tensors never queue
    # behind a big transfer they don't gate
    dma_plan = {
        'sync': ['ht_bf', 'htb_bf', 'wkv', 'sinsk', 'gk_rep', 'wiuq', 'wq',
                 'gq_rep', 'htq_r', 'bias'],
        'scalar': ['wccomp', 'cosk', 'gv_rep', 'wdq', 'coskb', 'htq', 'ww',
                   'cosq', 'sinsq', 'smaskT', 'a2'],
        'gpsimd': ['ident', 'ht', 'wcidx', 'iacc_tab', 'sinskb'],
    }
    cc = {}
    for qname, names in dma_plan.items():
        eng = getattr(nc, qname)
        for name in names:
            ap = dins[name]
            t = consts.tile(list(ap.shape), cdt.get(name, F32), tag=f"c_{name}")
            eng.dma_start(out=t, in_=ap)
            cc[name] = t
    ht, ht_bf, htq, htb_bf = cc['ht'], cc['ht_bf'], cc['htq'], cc['htb_bf']
    htq_r = cc['htq_r']
    smaskT, iacc_tab = cc['smaskT'], cc['iacc_tab']
    cosq, sinsq, gq_rep = cc['cosq'], cc['sinsq'], cc['gq_rep']
    cosk, sinsk = cc['cosk'], cc['sinsk']
    coskb, sinskb = cc['coskb'], cc['sinskb']
    gk_rep, gv_rep = cc['gk_rep'], cc['gv_rep']
    wdq, wiuq, ww, wcidx = cc['wdq'], cc['wiuq'], cc['ww'], cc['wcidx']
    wccomp, wkv, wq, a2_sb = cc['wccomp'], cc['wkv'], cc['wq'], cc['a2']
    bias, ident = cc['bias'], cc['ident']

    if dbg is not None:
        pass
    eps_t = consts.tile([128, 1], F32)
    nc.vector.memset(eps_t, EPS)

    # persistent intermediates
    vc = persist.tile([128, 4, C + 1], BF16)      # compressed V [s, c] + ones
    vb = persist.tile([128, 8, C + 1], BF16)      # band V + ones
    ktc = persist.tile([64, TC], F32R)            # compressed K^T [c, s]
    ktb = persist.tile([64, NBAND], BF16)         # band K^T
    qt = persist.tile([64, NH, TQ], F32R)         # Q^T per head
    qt_bf = persist.tile([64, NH, TQ], BF16)      # Q^T bf16 (band)
    hdct = persist.tile([64, TQ], F32R)           # H_dc^T
    qit = persist.tile([32, NH, TQ], F32R)        # Q_I^T per indexer head
    kit = persist.tile([32, TC], F32R)            # K_I^T
    wiw = persist.tile([128, NQT, 4], F32)        # indexer head weights
    selmask = persist.tile([128, NQT, TC], F32)   # top-8 mask, [q, s] layout
    selmaskT = persist.tile([128, 4, TQ], F32)    # transposed [s, (k, q)]
    o_all = persist.tile([128, NQT, NH, C + 1], F32)
    recip = persist.tile([128, NQT, NH], F32)
    ptc0 = persist.tile([128, NH, TQ], BF16)      # comp P~ for k=0 per head
    ptc1 = persist.tile([128, NH, 384], BF16)     # k=1
    ptc2 = persist.tile([128, 2, 2, 256], BF16)   # k=2, head pairs
    ptc3 = persist.tile([128, NH, 128], BF16)     # k=3, all heads
    pbt = persist.tile([128, 8, NH, 128], BF16)   # band P~ per (i,hb) all heads

    # ones column of the augmented V
    nc.sync.dma_start(out=vc[:, :, C:], in_=dins['ones_vc'])
    nc.sync.dma_start(out=vb[:, :, C:], in_=dins['ones_vb'])

    def strided_ht(src, dd, off, count):
        base = src[:, dd, :]
        return bass.AP(tensor=base.tensor, offset=base.offset + off,
                       ap=[base.ap[0], [4, count]])

    # ---------------- Stage A: compressed + band KV -> K^T, V ----------------
    def rep_ap(t, inner, nseg=4):
        """broadcast AP: [128, inner-table] read nseg times."""
        return bass.AP(tensor=t.tensor, offset=t.offset,
                       ap=[t.ap[0], [0, nseg], [1, inner]])

    def kv_group(kv_sb, ct, st, kout, vout, nseg=4):
        """norm+rope nseg KV tiles at once: kv_sb [128, nseg, 64] -> kout/vout.
        ct: [128, nseg, 32] cos table; st: [128, nseg, 64] signed sin."""
        W = nseg * C
        ct2 = bass.AP(tensor=ct.tensor, offset=ct.offset,
                      ap=[ct.ap[0], [32, nseg], [0, 2], [1, 32]])
        st2 = st
        sq = scr.tile([128, W], F32, tag="g_sq")
        nc.vector.tensor_mul(sq, kv_sb, kv_sb)
        ssum = scr.tile([128, nseg], F32, tag="g_ssum")
        nc.vector.reduce_sum(ssum, sq.rearrange("p (s c) -> p s c", s=nseg),
                             axis=mybir.AxisListType.X)
        den = scr.tile([128, nseg], F32, tag="g_den")
        nc.scalar.activation(den, ssum, AF.Sqrt, bias=eps_t, scale=1.0 / C)
        r4 = scr.tile([128, nseg], F32, tag="g_r4")
        nc.vector.reciprocal(r4, den)
        kn = scr.tile([128, W], F32, tag="g_kn")
        nc.vector.tensor_mul(kn, kv_sb,
                             bass.AP(tensor=r4.tensor, offset=r4.offset,
                                     ap=[r4.ap[0], [1, nseg], [0, C]]))
        yk = scr.tile([128, W], F32, tag="g_yk")
        nc.gpsimd.tensor_mul(yk, kn, rep_ap(gk_rep, C, nseg))
        yks = bass.AP(tensor=yk.tensor, offset=yk.offset + 32,
                      ap=[yk.ap[0], [64, nseg], [-32, 2], [1, 32]])
        t1 = scr.tile([128, W], F32, tag="g_t1")
        t2 = scr.tile([128, W], F32, tag="g_t2")
        nc.gpsimd.tensor_mul(t1, yk, ct2)
        nc.gpsimd.tensor_mul(t2, yks, st2)
        nc.gpsimd.tensor_add(kout, t1, t2)
        yv = scr.tile([128, W], F32, tag="g_yv")
        nc.vector.tensor_mul(yv, kn, rep_ap(gv_rep, C, nseg))
        yvs = bass.AP(tensor=yv.tensor, offset=yv.offset + 32,
                      ap=[yv.ap[0], [64, nseg], [-32, 2], [1, 32]])
        t3 = scr.tile([128, W], F32, tag="g_t3")
        t4 = scr.tile([128, W], F32, tag="g_t4")
        nc.vector.tensor_mul(t3, yv, ct2)
        nc.vector.tensor_mul(t4, yvs, st2)
        nc.vector.tensor_add(vout, t3, t4)

    # A1: compressed KV^T (pre-norm) via wide-N streaming, then transpose
    pkvc = ps_big.tile([64, TC], F32, tag="pbig")
    for step in range(16):
        j, dd = divmod(step, 2)
        nc.tensor.matmul(pkvc, wccomp[:, step, :],
                         strided_ht(ht_bf, dd, j, TC),
                         start=(step == 0), stop=(step == 15))
    kvct_sb = persist.tile([64, TC], F32, tag="kvct_sb")
    nc.vector.tensor_copy(kvct_sb, pkvc)
    kvc_sb = persist.tile([128, 4, C], F32, tag="kvc_sb")
    for k in range(4):
        ptp = ps_tp.tile([128, 128], F32, tag="ptp")
        nc.tensor.transpose(ptp[:, :64], kvct_sb[:, 128 * k:128 * (k + 1)],
                            ident[:64, :64])
        nc.scalar.copy(kvc_sb[:, k, :], ptp[:, :64])
    kc_all = persist.tile([128, 4, C], F32, tag="kc_all")
    kv_group(kvc_sb, cosk, sinsk, kc_all, vc[:, :, :C])
    for si in range(4):
        ptr = ps_tp.tile([64, 128], F32, tag="ptp")
        nc.tensor.transpose(ptr, kc_all[:, si, :], ident)
        if si % 2 == 0:
            nc.vector.tensor_copy(ktc[:, 128 * si:128 * (si + 1)], ptr)
        else:
            nc.scalar.copy(ktc[:, 128 * si:128 * (si + 1)], ptr)

    # A2: band KV (small-N; batching not worth it)
    kvb_sb = persist.tile([128, 8, C], F32, tag="kvb_sb")
    for si in range(8):
        pkv = ps_mid.tile([128, C], F32, tag="pmid")
        for dd in range(2):
            nc.tensor.matmul(pkv, htb_bf[:, dd, 128 * si:128 * (si + 1)],
                             wkv[:, dd, :], start=(dd == 0), stop=(dd == 1))
        nc.scalar.copy(kvb_sb[:, si, :], pkv)
    kb_all = persist.tile([128, 8, C], F32, tag="kb_all")
    for g in range(2):
        kv_group(kvb_sb[:, 4 * g:4 * (g + 1), :],
                 coskb[:, 4 * g:4 * (g + 1), :], sinskb[:, 4 * g:4 * (g + 1), :],
                 kb_all[:, 4 * g:4 * (g + 1), :], vb[:, 4 * g:4 * (g + 1), :C])
    for si in range(8):
        ptr = ps_tp.tile([64, 128], F32, tag="ptp")
        nc.tensor.transpose(ptr, kb_all[:, si, :], ident)
        if si % 2 == 0:
            nc.vector.tensor_copy(ktb[:, 128 * si:128 * (si + 1)], ptr)
        else:
            nc.scalar.copy(ktb[:, 128 * si:128 * (si + 1)], ptr)

    # ---------------- Stage B: indexer ----------------
    pkit = ps_big.tile([32, TC], F32, tag="pbig")
    for step in range(16):
        j, dd = divmod(step, 2)
        nc.tensor.matmul(pkit, wcidx[:, step, :], strided_ht(ht, dd, j, TC),
                         start=(step == 0), stop=(step == 15))
    nc.vector.tensor_copy(kit, pkit)

    phdc = ps_big.tile([64, TQ], F32, tag="pbig")
    for dd in range(2):
        nc.tensor.matmul(phdc, wdq[:, dd, :], htq[:, dd, :],
                         start=(dd == 0), stop=(dd == 1))
    nc.vector.tensor_copy(hdct, phdc)

    pqit = ps_big.tile([128, TQ], F32, tag="pbig")
    nc.tensor.matmul(pqit, wiuq, hdct, start=True, stop=True)
    for h in range(NH):
        nc.scalar.copy(qit[:, h, :], pqit[32 * h:32 * (h + 1), :])

    for i in range(NQT):
        pwiw = ps_mid.tile([128, 4], F32, tag="pmid")
        nc.tensor.matmul(pwiw, hdct[:, 128 * i:128 * (i + 1)], ww,
                         start=True, stop=True)
        nc.scalar.copy(wiw[:, i, :], pwiw)

    for i in range(NQT):
        # full-width dots (invalid region swamped by the -1e30 table ramp)
        iacc = scr_big.tile([128, TC], F32, tag="iacc")
        for h in range(4):
            pdot = ps_big.tile([128, TC], F32, tag="pbig")
            nc.tensor.matmul(pdot, qit[:, h, 128 * i:128 * (i + 1)],
                             kit, start=True, stop=True)
            tmp = scr_big.tile([128, TC], F32, tag="tmp")
            nc.vector.tensor_scalar(tmp, pdot, 0.0, wiw[:, i, h:h + 1],
                                    op0=ALU.max, op1=ALU.mult)
            if h == 0:
                nc.gpsimd.tensor_add(iacc, tmp, iacc_tab[:, i, :])
            else:
                nc.gpsimd.tensor_add(iacc, iacc, tmp)
        top8 = scr.tile([128, 8], F32, tag="top8")
        nc.vector.max(out=top8, in_=iacc)
        seng = nc.gpsimd if i % 2 == 0 else nc.vector
        seng.tensor_scalar(selmask[:, i, :], iacc, top8[:, 7:8], None,
                           op0=ALU.is_ge)
        for k in range(i + 1):
            ptp = ps_tp.tile([128, 128], F32, tag="ptp")
            nc.tensor.transpose(ptp, selmask[:, i, 128 * k:128 * (k + 1)], ident)
            nc.scalar.copy(selmaskT[:, k, 128 * i:128 * (i + 1)], ptp)

    # ---------------- Stage C: Q ----------------
    for i in range(NQT):
        pq = ps_mid.tile([128, 256], F32, tag="pmid")
        for dd in range(2):
            nc.tensor.matmul(pq, htq_r[:, dd, 128 * i:128 * (i + 1)],
                             wq[:, dd, :], start=(dd == 0), stop=(dd == 1))
        q_sb = scr.tile([128, 256], F32, tag="q_sb")
        nc.scalar.copy(q_sb, pq)
        sq = scr.tile([128, 256], F32, tag="q_sq")
        nc.vector.tensor_mul(sq, q_sb, q_sb)
        ssum = scr.tile([128, 4], F32, tag="q_ssum")
        nc.vector.reduce_sum(ssum, sq.rearrange("p (h c) -> p h c", h=4),
                             axis=mybir.AxisListType.X)
        den = scr.tile([128, 4], F32, tag="q_den")
        nc.scalar.activation(den, ssum, AF.Sqrt, bias=eps_t, scale=1.0 / C)
        r4 = scr.tile([128, 4], F32, tag="q_r4")
        nc.vector.reciprocal(r4, den)
        qg = scr.tile([128, 256], F32, tag="q_g")
        nc.vector.tensor_mul(qg, q_sb, gq_rep)
        qn = scr.tile([128, 256], F32, tag="q_n")
        nc.vector.tensor_mul(qn, qg,
                             bass.AP(tensor=r4.tensor, offset=r4.offset,
                                     ap=[r4.ap[0], [1, 4], [0, C]]))
        qns = bass.AP(tensor=qn.tensor, offset=qn.offset + 32,
                      ap=[qn.ap[0], [64, 4], [-32, 2], [1, 32]])
        cos_i = bass.AP(tensor=cosq.tensor, offset=cosq.offset + i * 32,
                        ap=[cosq.ap[0], [0, 4], [0, 2], [1, 32]])
        sins_i = bass.AP(tensor=sinsq.tensor, offset=sinsq.offset + i * C,
                         ap=[sinsq.ap[0], [0, 4], [1, C]])
        av = scr.tile([128, 256], F32, tag="q_a")
        bv = scr.tile([128, 256], F32, tag="q_b")
        nc.vector.tensor_mul(av, qn, cos_i)
        nc.vector.tensor_mul(bv, qns, sins_i)
        qrope = scr.tile([128, 256], F32, tag="qrope")
        nc.vector.tensor_add(qrope, av, bv)
        for h in range(4):
            ptq = ps_tp.tile([64, 128], F32, tag="ptp")
            nc.tensor.transpose(ptq, qrope[:, 64 * h:64 * (h + 1)], ident)
            nc.scalar.copy(qt[:, h, 128 * i:128 * (i + 1)], ptq)
            nc.vector.tensor_copy(qt_bf[:, h, 128 * i:128 * (i + 1)], ptq)

    # ---------------- Stage D: attention in S^T layout ----------------
    # D-S: all P~ tiles (comp k0/k1 per head; k2 head pairs; k3 all heads)
    for h in range(NH):
        for k, w in ((0, 512), (1, 384)):
            st_ps = ps_big.tile([128, TQ], F32, tag="pbig")
            nc.tensor.matmul(st_ps[:, :w], ktc[:, 128 * k:128 * (k + 1)],
                             qt[:, h, 128 * k:], start=True, stop=True)
            pexp = scr_big.tile([128, TQ], F32, tag="pexp")
            nc.scalar.activation(pexp[:, :w], st_ps[:, :w], AF.Exp, scale=0.125)
            dst = ptc0[:, h, :] if k == 0 else ptc1[:, h, :]
            eng = nc.vector if h % 2 == 0 else nc.gpsimd
            eng.tensor_mul(dst, pexp[:, :w], selmaskT[:, k, 128 * k:])
    for g in range(2):
        st_ps = ps_big.tile([128, TQ], F32, tag="pbig")
        nc.tensor.matmul(st_ps, ktc[:, 256:384], qt[:, 2 * g:2 * (g + 1), 256:],
                         start=True, stop=True)
        pexp = scr_big.tile([128, TQ], F32, tag="pexp")
        nc.scalar.activation(pexp, st_ps, AF.Exp, scale=0.125)
        eng = nc.vector if g == 0 else nc.gpsimd
        eng.tensor_mul(ptc2[:, g, :, :], pexp.rearrange("p (a b) -> p a b", a=2),
                       _rep(selmaskT[:, 2, 256:], 2))
    st_ps = ps_big.tile([128, TQ], F32, tag="pbig")
    nc.tensor.matmul(st_ps, ktc[:, 384:], qt[:, :, 384:], start=True, stop=True)
    pexp = scr_big.tile([128, TQ], F32, tag="pexp")
    nc.scalar.activation(pexp, st_ps, AF.Exp, scale=0.125)
    nc.vector.tensor_mul(ptc3, pexp.rearrange("p (a b) -> p a b", a=4),
                         _rep(selmaskT[:, 3, 384:], 4))

    # band S for all heads per (i, hb)
    for i in range(NQT):
        which = 0 if i == 0 else 1
        for hb in range(2):
            sb_ps = ps_big.tile([128, TQ], F32, tag="pbig")
            nc.tensor.matmul(sb_ps,
                             ktb[:, 256 * i + 128 * hb:256 * i + 128 * hb + 128],
                             qt_bf[:, :, 128 * i:128 * (i + 1)],
                             start=True, stop=True)
            pexpb = scr_big.tile([128, TQ], F32, tag="pexpb")
            nc.scalar.activation(pexpb, sb_ps, AF.Exp, scale=0.125)
            beng = nc.vector if (i + hb) % 2 == 0 else nc.gpsimd
            beng.tensor_mul(pbt[:, 2 * i + hb, :, :],
                            pexpb.rearrange("p (a b) -> p a b", a=4),
                            _rep(smaskT[:, hb, which, :], 4))

    # D-PV: per head accumulate O^T, evacuate, transpose
    for h in range(NH):
        oT = ps_ot.tile([C + 1, TQ], F32, tag="oT")
        comp_rhs = [ptc0[:, h, :], ptc1[:, h, :],
                    ptc2[:, h // 2, h % 2, :], ptc3[:, h, :]]
        for k in range(4):
            nc.tensor.matmul(oT[:, 128 * k:], vc[:, k, :], comp_rhs[k],
                             start=(k == 0), stop=False, skip_group_check=True)
        for i in range(NQT):
            for hb in range(2):
                nc.tensor.matmul(oT[:, 128 * i:128 * (i + 1)], vb[:, 2 * i + hb, :],
                                 pbt[:, 2 * i + hb, h, :], start=False,
                                 stop=(i == NQT - 1 and hb == 1),
                                 skip_group_check=True)
        oT_sb = scr.tile([C + 1, TQ], F32, tag="oT_sb")
        nc.scalar.copy(oT_sb, oT)
        for i in range(NQT):
            pto = ps_tp.tile([128, C + 1], F32, tag="ptp")
            nc.tensor.transpose(pto, oT_sb[:, 128 * i:128 * (i + 1)],
                                ident[:C + 1, :C + 1])
            if i % 2 == 0:
                nc.scalar.copy(o_all[:, i, h, :], pto)
            else:
                nc.vector.tensor_copy(o_all[:, i, h, :], pto)
            nc.vector.reciprocal(recip[:, i, h:h + 1], o_all[:, i, h, C:])

    # ---------------- Stage E: inverse rope + output projection ----------------
    for i in range(NQT):
        ox = bass.AP(tensor=o_all.tensor,
                     offset=o_all.offset + i * NH * (C + 1),
                     ap=[o_all.ap[0], [C + 1, 4], [1, C]])
        rb = bass.AP(tensor=recip.tensor, offset=recip.offset + i * NH,
                     ap=[recip.ap[0], [1, 4], [0, C]])
        on = scr.tile([128, 256], F32, tag="o_n")
        nc.vector.tensor_mul(on, ox, rb)
        ons = bass.AP(tensor=on.tensor, offset=on.offset + 32,
                      ap=[on.ap[0], [64, 4], [-32, 2], [1, 32]])
        cos_i = bass.AP(tensor=cosq.tensor, offset=cosq.offset + i * 32,
                        ap=[cosq.ap[0], [0, 4], [0, 2], [1, 32]])
        sins_i = bass.AP(tensor=sinsq.tensor, offset=sinsq.offset + i * C,
                         ap=[sinsq.ap[0], [0, 4], [1, C]])
        av = scr.tile([128, 256], F32, tag="o_a")
        bv = scr.tile([128, 256], F32, tag="o_b")
        nc.vector.tensor_mul(av, on, cos_i)
        nc.vector.tensor_mul(bv, ons, sins_i)
        orope = scr.tile([128, 256], F32, tag="orope")
        nc.vector.tensor_sub(orope, av, bv)
        ot2 = scr.tile([128, 2, 128], F32R, tag="ot2")
        for g in range(2):
            # one 128-wide transpose packs the head pair: out partition
            # p = 64*h' + c for heads (2g, 2g+1), matching a2's layout
            pto2 = ps_tp.tile([128, 128], F32, tag="ptp")
            nc.tensor.transpose(pto2, orope[:, 128 * g:128 * (g + 1)], ident)
            nc.scalar.copy(ot2[:, g, :], pto2)
        pout = ps_mid.tile([128, 256], F32, tag="pmid")
        for g in range(2):
            nc.tensor.matmul(pout, ot2[:, g, :], a2_sb[:, g, :],
                             start=(g == 0), stop=(g == 1))
        out_t = scr.tile([128, 256], F32, tag="out_t")
        nc.vector.tensor_add(out_t, pout, bias)
        nc.sync.dma_start(out=out_d[:, i, :], in_=out_t)

    if dbg is not None:
        nc.sync.dma_start(out=dbg['selmask'], in_=selmask)
        nc.sync.dma_start(out=dbg['kit'], in_=kit)
        nc.sync.dma_start(out=dbg['hdct'], in_=hdct)
        nc.sync.dma_start(out=dbg['qit'], in_=qit)
        nc.sync.dma_start(out=dbg['wiw'], in_=wiw)

    ctx.close()


# ---------------------------------------------------------------------------
# Host-side input preparation
# ---------------------------------------------------------------------------

def _rope_tables(pos, g=None, inverse=False):
    half = C // 2
    inv_freq = (1.0 / (10000.0 ** (np.arange(half, dtype=np.float32) / half)))
    ang = pos.astype(np.float32)[:, None] * inv_freq[None, :]
    if inverse:
        ang = -ang
    cos, sin = np.cos(ang), np.sin(ang)
    ctab = np.concatenate([cos, cos], axis=1)
    stab = np.concatenate([-sin, sin], axis=1)
    if g is not None:
        gswap = np.concatenate([g[half:], g[:half]])
        ctab = ctab * g[None, :]
        stab = stab * gswap[None, :]
    return ctab.astype(np.float32), stab.astype(np.float32)


def _tile_rows(x, ntiles):
    n, f = x.shape
    assert n == ntiles * 128
    return np.ascontiguousarray(x.reshape(ntiles, 128, f).transpose(1, 0, 2))


def _qpos(j):
    """Global query positions of core-chunk j (interleaved tiles g=4i+j)."""
    return np.concatenate([128 * (4 * i + j) + np.arange(128) for i in range(NQT)])


def _prep_core_inputs(inputs, core):
    H = np.asarray(inputs['H'], np.float32)
    g_q = np.asarray(inputs['g_q'], np.float32)
    g_k = np.asarray(inputs['g_k'], np.float32)
    g_v = np.asarray(inputs['g_v'], np.float32)

    b, j = divmod(core, 4)
    HT = H[b].T                                     # (256, 2048)
    tq = _qpos(j)                                   # (512,) global query positions

    d = {}
    ht = np.zeros((256, TPAD), np.float32)
    ht[:, :T] = HT
    d['ht'] = np.ascontiguousarray(ht.reshape(2, 128, TPAD).transpose(1, 0, 2))
    d['ht_bf'] = d['ht'].astype(ml_dtypes.bfloat16)
    d['htq'] = np.ascontiguousarray(
        HT[:, tq].reshape(2, 128, TQ).transpose(1, 0, 2))
    d['htq_r'] = d['htq']

    # band cols: per local tile i, global tile g=4i+j, band t in [128g-128, 128g+128)
    htb = np.zeros((256, NBAND), np.float32)
    for i in range(NQT):
        t0 = 128 * (4 * i + j)
        lo = t0 - 128
        src_lo = max(lo, 0)
        htb[:, 256 * i + (src_lo - lo):256 * i + (t0 + 128 - lo)] = HT[:, src_lo:t0 + 128]
    d['htb_bf'] = np.ascontiguousarray(
        htb.reshape(2, 128, NBAND).transpose(1, 0, 2)).astype(ml_dtypes.bfloat16)

    # additive causal/tie ramp table: valid (4s <= t): -s*1e-30 (tie ramp),
    # invalid: -1e30 - s*1e24 (-inf leakage ordering for rows w/ <8 blocks)
    tmat = tq.reshape(NQT, 128).T.astype(np.int64)  # (128, NQT)
    s = np.arange(TC, dtype=np.int64)
    sf = s.astype(np.float32)
    rampv = sf * np.float32(-1e-30)
    rampi = sf * np.float32(-1e24) + np.float32(-1e30)
    valid = (4 * s)[None, None, :] <= tmat[:, :, None]   # (128, NQT, TC)
    d['iacc_tab'] = np.where(valid, rampv[None, None, :],
                             rampi[None, None, :]).astype(np.float32)

    # transposed sliding masks: smaskT[s_local(2x128), which, q(128)]
    r = np.arange(128)[None, :]
    jj = np.arange(256)[:, None]
    base = ((jj >= r + 113) & (jj <= r + 128)).astype(np.float32)   # (256 s, 128 q)
    first = base.copy()
    if j == 0:
        first *= (jj >= 128)                        # s >= 0 for global tile 0
    sm = np.stack([first, base], axis=1)            # (256, 2 which, 128)
    d['smaskT'] = np.ascontiguousarray(
        sm.reshape(2, 128, 2, 128).transpose(1, 0, 2, 3)).astype(ml_dtypes.bfloat16)

    cq, sq_ = _rope_tables(tq)                      # (512, 64) each
    d['cosq'] = _tile_rows(np.ascontiguousarray(cq[:, :32]), NQT)
    d['sinsq'] = _tile_rows(sq_, NQT)
    d['gq_rep'] = np.broadcast_to(g_q.reshape(1, 256), (128, 256)).copy()

    ck, sk = _rope_tables(np.arange(TC))
    d['cosk'] = _tile_rows(np.ascontiguousarray(ck[:, :32]), 4)
    d['sinsk'] = _tile_rows(sk, 4)
    d['gk_rep'] = np.broadcast_to(g_k.reshape(1, C), (128, C)).copy()
    d['gv_rep'] = np.broadcast_to(g_v.reshape(1, C), (128, C)).copy()

    band_pos = np.concatenate(
        [TC + 128 * (4 * i + j) - 128 + np.arange(256) for i in range(NQT)])
    band_pos = np.maximum(band_pos, 0)              # padded rows are zero anyway
    ckb, skb = _rope_tables(band_pos)
    d['coskb'] = _tile_rows(np.ascontiguousarray(ckb[:, :32]), 8)
    d['sinskb'] = _tile_rows(skb, 8)
    return d


def _prep_shared_inputs(inputs):
    Wc_comp = np.asarray(inputs['Wc_comp'], np.float32)
    Wc_idx = np.asarray(inputs['Wc_idx'], np.float32)
    W_DQ = np.asarray(inputs['W_DQ'], np.float32)
    W_IUQ = np.asarray(inputs['W_IUQ'], np.float32)
    W_w = np.asarray(inputs['W_w'], np.float32)
    W_Q = np.asarray(inputs['W_Q'], np.float32)
    W_KV = np.asarray(inputs['W_KV'], np.float32)
    Wg0 = np.asarray(inputs['Wg0'], np.float32)
    bg0 = np.asarray(inputs['bg0'], np.float32)
    Wg1 = np.asarray(inputs['Wg1'], np.float32)
    bg1 = np.asarray(inputs['bg1'], np.float32)
    Wout = np.asarray(inputs['Wout'], np.float32)
    bout = np.asarray(inputs['bout'], np.float32)

    d = {}
    d['wdq'] = np.ascontiguousarray(W_DQ.reshape(2, 128, 64).transpose(1, 0, 2))
    d['wiuq'] = W_IUQ.copy()
    d['ww'] = W_w.copy()
    d['wcidx'] = np.ascontiguousarray(
        Wc_idx.reshape(8, 2, 128, 32).transpose(2, 0, 1, 3).reshape(128, 16, 32))
    d['wccomp'] = np.ascontiguousarray(
        Wc_comp.reshape(8, 2, 128, C).transpose(2, 0, 1, 3).reshape(128, 16, C)
    ).astype(ml_dtypes.bfloat16)
    d['wkv'] = np.ascontiguousarray(
        W_KV.reshape(2, 128, C).transpose(1, 0, 2)).astype(ml_dtypes.bfloat16)
    d['wq'] = np.ascontiguousarray(W_Q.reshape(2, 128, 256).transpose(1, 0, 2))
    A = np.stack([Wg0[:64] @ Wout[:64], Wg0[64:] @ Wout[:64],
                  Wg1[:64] @ Wout[64:], Wg1[64:] @ Wout[64:]], axis=0)
    # head-pair packing: partitions 0-63 = head 2g, 64-127 = head 2g+1
    d['a2'] = np.ascontiguousarray(
        np.stack([np.vstack([A[0], A[1]]), np.vstack([A[2], A[3]])], axis=1))
    bias_v = bout + bg0 @ Wout[:64] + bg1 @ Wout[64:]
    d['bias'] = np.broadcast_to(bias_v.astype(np.float32), (128, 256)).copy()
    d['ident'] = np.eye(128, dtype=np.float32)
    d['ones_vc'] = np.ones((128, 4, 1), ml_dtypes.bfloat16)
    d['ones_vb'] = np.ones((128, 8, 1), ml_dtypes.bfloat16)
    return d


def make_in_maps(inputs):
    shared = _prep_shared_inputs(inputs)
    maps = []
    for core in range(8):
        m = dict(shared)
        m.update(_prep_core_inputs(inputs, core))
        maps.append(m)
    return maps


def gather_output(results):
    """results: list of 8 per-core dicts with 'out' (128, 4, 256)."""
    out = np.zeros((B, T, D), np.float32)
    for core in range(8):
        b, j = divmod(core, 4)
        o = np.asarray(results[core]["out"])
        for i in range(NQT):
            g = 4 * i + j
            out[b, 128 * g:128 * (g + 1)] = o[:, i, :]
    return out


_NC_CACHE = None


def kernel(**inputs):
    global _NC_CACHE
    if _NC_CACHE is None:
        _NC_CACHE = build_program()
    in_maps = make_in_maps(inputs)
    res = run_bass_kernel_spmd(_NC_CACHE, in_maps, core_ids=list(range(8)))
    return gather_output(res.results)
